# revision 1
# baseline (speedup 1.0000x reference)
"""DecoderRNN (LSTMCell + Linear over 256 steps) on 8 trn2 NeuronCores.

Data-parallel on batch (64 -> 8 per core), weights replicated. Per core:
  P1: pregates[T*8, 4H] = x @ W_ih.T + (b_ih + b_hh)  (GEMM, spilled to DRAM)
  P2: 256 sequential LSTM steps. The PE streams W_hh.T (float32r, 1 col/cycle)
      against stationary h.T tiles; gate chunks are consumed bank-by-bank by
      DVE/ACT while later chunks still stream. Chunk order [1,3,5,7,0,2,4,6]
      finishes the h1 chain early so the next step's k=4..7 matmuls fill the
      PE while the h0 tail completes.
  P3: out[T*8, H] = h_all @ W_fc.T + b_fc             (GEMM)

Host-side: inputs are sliced/transposed per core; per-core outputs
[T*8(H-major rows (t,b)), H] are reassembled to [64, T, H].
"""

import sys

for _p in ("/opt/trn_rl_repo", "/root/.axon_site/_ro/trn_rl_repo"):
    if _p not in sys.path:
        sys.path.insert(0, _p)

import numpy as np

import bass_rust
from concourse import bass, tile, mybir

F32 = mybir.dt.float32
F32R = mybir.dt.float32r
AF = mybir.ActivationFunctionType

E = 512
H = 1024
G4 = 4 * H
B_LOC = 8
T = 256
BT = T * B_LOC
M_TILES = BT // 128
STEPS_PER_MT = 128 // B_LOC


def _split_multi_waits(nc, max_waits=1):
    """walrus CoreV3 codegen rejects >1 sync-wait per instruction; split
    extras onto preceding single-wait NoOps on the same engine."""
    for f in nc.m.functions:
        for blk in f.blocks:
            insts = list(blk.instructions)
            out = []
            dirty = False
            for inst in insts:
                si = inst.sync_info
                if si is not None and len(si.on_wait) > max_waits:
                    waits = list(si.on_wait)
                    for j, w in enumerate(waits[:-max_waits]):
                        nop = bass_rust.InstNoOp(
                            name=f"{inst.name}-sw{j}", ins=[], outs=[]
                        )
                        nop.engine = inst.engine
                        nop.sync_info = bass_rust.SyncInfo(on_wait=[w], on_update=[])
                        out.append(nop)
                    inst.sync_info = bass_rust.SyncInfo(
                        on_wait=waits[-max_waits:], on_update=list(si.on_update)
                    )
                    dirty = True
                out.append(inst)
            if dirty:
                blk.instructions = out


def build_nc():
    nc = bass.Bass()

    xT = nc.dram_tensor("xT", [E, BT], F32R, kind="ExternalInput")
    wihT = nc.dram_tensor("wihT", [E, G4], F32R, kind="ExternalInput")
    whhT = nc.dram_tensor("whhT", [H, G4], F32R, kind="ExternalInput")
    wfcT = nc.dram_tensor("wfcT", [H, H], F32R, kind="ExternalInput")
    biasg = nc.dram_tensor("biasg", [128, G4], F32, kind="ExternalInput")
    biasf = nc.dram_tensor("biasf", [128, H], F32, kind="ExternalInput")
    ident = nc.dram_tensor("ident", [B_LOC, B_LOC], F32, kind="ExternalInput")
    hzero = nc.dram_tensor("hzero", [128, B_LOC], F32R, kind="ExternalInput")
    out = nc.dram_tensor("out", [BT, H], F32, kind="ExternalOutput")

    with tile.TileContext(nc) as tc:
        with tc.tile_pool(name="dram", bufs=1, space="DRAM") as dpool:
            pregates = dpool.tile([BT, G4], F32)
            hallT = [dpool.tile([128, BT], F32R, name=f"hallT{k}", tag=f"hallT{k}") for k in range(8)]

            # ---- P1: pregates = x @ W_ih.T + (b_ih + b_hh) ----
            with (
                tc.tile_pool(name="p1", bufs=1) as p1,
                tc.tile_pool(name="p1ps", bufs=1, space="PSUM") as p1ps,
            ):
                xT_sb = [p1.tile([128, BT], F32R, name=f"xT{k}", tag=f"xT{k}") for k in range(4)]
                wihT_sb = [p1.tile([128, G4], F32R, name=f"wih{k}", tag=f"wih{k}") for k in range(4)]
                bg_sb = p1.tile([128, G4], F32, name="bg", tag="bg")
                for k in range(4):
                    nc.sync.dma_start(out=xT_sb[k][:], in_=xT[128 * k : 128 * (k + 1), :])
                    nc.sync.dma_start(out=wihT_sb[k][:], in_=wihT[128 * k : 128 * (k + 1), :])
                nc.sync.dma_start(out=bg_sb[:], in_=biasg[:])

                for m in range(M_TILES):
                    ps = [p1ps.tile([128, 512], F32, name=f"ps{n}", tag=f"ps{n}", bufs=1) for n in range(8)]
                    for k in range(4):
                        lhsT = xT_sb[k][:, 128 * m : 128 * (m + 1)]
                        for n in range(8):
                            nc.tensor.matmul(
                                ps[n][:],
                                lhsT,
                                wihT_sb[k][:, 512 * n : 512 * (n + 1)],
                                start=(k == 0),
                                stop=(k == 3),
                            )
                    pre_m = p1.tile([128, G4], F32, name="prem", tag="prem", bufs=3)
                    for n in range(8):
                        nc.vector.tensor_add(
                            pre_m[:, 512 * n : 512 * (n + 1)],
                            ps[n][:],
                            bg_sb[:, 512 * n : 512 * (n + 1)],
                        )
                    nc.sync.dma_start(
                        out=pregates[128 * m : 128 * (m + 1), :], in_=pre_m[:]
                    )

            # ---- P2: the recurrence ----
            with (
                tc.tile_pool(name="p2", bufs=1) as p2,
                tc.tile_pool(name="p2ps", bufs=1, space="PSUM") as p2ps,
            ):
                whh_sb = [p2.tile([128, G4], F32R, name=f"whh{k}", tag=f"whh{k}") for k in range(8)]
                for k in [4, 5, 6, 7, 0, 1, 2, 3]:
                    nc.sync.dma_start(out=whh_sb[k][:], in_=whhT[128 * k : 128 * (k + 1), :])
                id_sb = p2.tile([B_LOC, B_LOC], F32, name="id8", tag="id8")
                nc.sync.dma_start(out=id_sb[:], in_=ident[:])

                c = p2.tile([B_LOC, H], F32, name="c", tag="c", bufs=1)
                nc.vector.memset(c[:], 0.0)
                hz_sb = p2.tile([128, B_LOC], F32R, name="hz", tag="hz", bufs=1)
                nc.sync.dma_start(out=hz_sb[:], in_=hzero[:])
                hT_prev = [hz_sb[:] for _ in range(8)]

                # h1's full chain (i1,f1,g1 -> c1; o1 -> h1) completes early so
                # next step's k=4..7 matmuls start the moment this stream ends.
                CHUNK_ORDER = [1, 3, 5, 7, 0, 2, 4, 6]
                ACT_FN = {0: AF.Sigmoid, 1: AF.Sigmoid, 2: AF.Sigmoid, 3: AF.Sigmoid,
                          4: AF.Tanh, 5: AF.Tanh, 6: AF.Sigmoid, 7: AF.Sigmoid}
                K_ORDER = [4, 5, 6, 7, 0, 1, 2, 3]

                hstg = None
                for t in range(T):
                    mt, j = divmod(t, STEPS_PER_MT)
                    if j == 0:
                        hstg = [
                            p2.tile([128, 128], F32R, name=f"hstg{k}", tag=f"hstg{k}", bufs=2)
                            for k in range(8)
                        ]
                    pre_t = p2.tile([B_LOC, G4], F32, name="pret", tag="pret", bufs=2)
                    nc.sync.dma_start(
                        out=pre_t[:], in_=pregates[B_LOC * t : B_LOC * (t + 1), :]
                    )

                    a = {}
                    tc_half = {}
                    hT_new = [None] * 8
                    for n in CHUNK_ORDER:
                        ps_n = p2ps.tile([B_LOC, 512], F32, name="gps", tag="gps", bufs=4)
                        for ki, k in enumerate(K_ORDER):
                            nc.tensor.matmul(
                                ps_n[:],
                                hT_prev[k],
                                whh_sb[k][:, 512 * n : 512 * (n + 1)],
                                start=(ki == 0),
                                stop=(ki == 7),
                            )
                        a_n = p2.tile([B_LOC, 512], F32, name=f"a{n}", tag=f"a{n}", bufs=1)
                        nc.vector.tensor_add(
                            a_n[:], ps_n[:], pre_t[:, 512 * n : 512 * (n + 1)]
                        )
                        nc.scalar.activation(a_n[:], a_n[:], ACT_FN[n])
                        a[n] = a_n

                        if n in (4, 5):  # g-chunk done -> c update for this half
                            hh = n - 4
                            csl = c[:, 512 * hh : 512 * (hh + 1)]
                            t1 = p2.tile([B_LOC, 512], F32, name="t1", tag="t1", bufs=1)
                            nc.vector.tensor_mul(t1[:], a[2 + hh][:], csl)
                            t2 = p2.tile([B_LOC, 512], F32, name="t2", tag="t2", bufs=1)
                            nc.vector.tensor_mul(t2[:], a[hh][:], a_n[:])
                            nc.vector.tensor_add(csl, t1[:], t2[:])
                            tch = p2.tile([B_LOC, 512], F32, name=f"tc{hh}", tag=f"tc{hh}", bufs=1)
                            nc.scalar.activation(tch[:], csl, AF.Tanh)
                            tc_half[hh] = tch

                        if n in (6, 7):  # o-chunk done -> h half + transposes
                            hh = n - 6
                            hhf = p2.tile([B_LOC, 512], F32, name=f"h{hh}", tag=f"h{hh}", bufs=1)
                            nc.vector.tensor_mul(hhf[:], a_n[:], tc_half[hh][:])
                            for kk in range(4):
                                k = 4 * hh + kk
                                tp = p2ps.tile([128, B_LOC], F32, name="tps", tag="tps", bufs=4)
                                nc.tensor.transpose(
                                    tp[:], hhf[:, 128 * kk : 128 * (kk + 1)], id_sb[:]
                                )
                                dst = hstg[k][:, B_LOC * j : B_LOC * (j + 1)]
                                # h0 (k<4) lands at the step tail: its copy goes
                                # to the idle Scalar engine so next-step matmuls
                                # aren't stuck behind DVE
                                if k < 4:
                                    nc.scalar.copy(dst, tp[:])
                                else:
                                    nc.vector.tensor_copy(dst, tp[:])
                                hT_new[k] = dst

                    hT_prev = hT_new
                    if j == STEPS_PER_MT - 1:
                        for k in range(8):
                            nc.sync.dma_start(
                                out=hallT[k][:, 128 * mt : 128 * (mt + 1)],
                                in_=hstg[k][:],
                            )

            # ---- P3: out = h_all @ W_fc.T + b_fc ----
            with (
                tc.tile_pool(name="p3", bufs=1) as p3,
                tc.tile_pool(name="p3ps", bufs=1, space="PSUM") as p3ps,
            ):
                wfc_sb = [p3.tile([128, H], F32R, name=f"wfc{k}", tag=f"wfc{k}") for k in range(8)]
                for k in range(8):
                    nc.sync.dma_start(out=wfc_sb[k][:], in_=wfcT[128 * k : 128 * (k + 1), :])
                bf_sb = p3.tile([128, H], F32, name="bf", tag="bf")
                nc.sync.dma_start(out=bf_sb[:], in_=biasf[:])

                for m in range(M_TILES):
                    hm = [p3.tile([128, 128], F32R, name=f"hm{k}", tag=f"hm{k}", bufs=2) for k in range(8)]
                    for k in range(8):
                        nc.sync.dma_start(
                            out=hm[k][:], in_=hallT[k][:, 128 * m : 128 * (m + 1)]
                        )
                    ps = [p3ps.tile([128, 512], F32, name=f"p3ps{n}", tag=f"p3ps{n}", bufs=2) for n in range(2)]
                    for k in range(8):
                        for n in range(2):
                            nc.tensor.matmul(
                                ps[n][:],
                                hm[k][:],
                                wfc_sb[k][:, 512 * n : 512 * (n + 1)],
                                start=(k == 0),
                                stop=(k == 7),
                            )
                    osb = p3.tile([128, H], F32, name="osb", tag="osb", bufs=3)
                    for n in range(2):
                        nc.vector.tensor_add(
                            osb[:, 512 * n : 512 * (n + 1)],
                            ps[n][:],
                            bf_sb[:, 512 * n : 512 * (n + 1)],
                        )
                    nc.sync.dma_start(out=out[128 * m : 128 * (m + 1), :], in_=osb[:])

    _split_multi_waits(nc)
    return nc


def make_in_maps(embedded, W_ih, W_hh, b_ih, b_hh, W_fc, b_fc):
    cast = lambda x: np.ascontiguousarray(x, dtype=np.float32)
    wihT = cast(W_ih.T)
    whhT = cast(W_hh.T)
    wfcT = cast(W_fc.T)
    biasg = np.ascontiguousarray(
        np.broadcast_to((b_ih + b_hh).astype(np.float32), (128, G4))
    )
    biasf = np.ascontiguousarray(np.broadcast_to(b_fc.astype(np.float32), (128, H)))
    identm = np.eye(B_LOC, dtype=np.float32)
    hzero = np.zeros((128, B_LOC), dtype=np.float32)

    in_maps = []
    for core in range(8):
        xc = embedded[core * B_LOC : (core + 1) * B_LOC]  # [8, T, E]
        x2d = xc.transpose(1, 0, 2).reshape(T * B_LOC, E)  # rows (t, b)
        in_maps.append(
            {
                "xT": cast(x2d.T),
                "wihT": wihT,
                "whhT": whhT,
                "wfcT": wfcT,
                "biasg": biasg,
                "biasf": biasf,
                "ident": identm,
                "hzero": hzero,
            }
        )
    return in_maps


_CACHED_NC = None


def kernel(embedded, W_ih, W_hh, b_ih, b_hh, W_fc, b_fc):
    global _CACHED_NC
    from concourse.bass_utils import run_bass_kernel_spmd

    embedded = np.asarray(embedded, dtype=np.float32)
    if _CACHED_NC is None:
        _CACHED_NC = build_nc()
    in_maps = make_in_maps(
        embedded,
        np.asarray(W_ih, np.float32), np.asarray(W_hh, np.float32),
        np.asarray(b_ih, np.float32), np.asarray(b_hh, np.float32),
        np.asarray(W_fc, np.float32), np.asarray(b_fc, np.float32),
    )
    res = run_bass_kernel_spmd(_CACHED_NC, in_maps, list(range(8)))
    outs = []
    for core in range(8):
        o = res.results[core]["out"]  # [T*8, H] rows (t, b)
        outs.append(o.reshape(T, B_LOC, H).transpose(1, 0, 2))
    return np.concatenate(outs, axis=0)  # [64, T, H] float32
